# revision 32
# baseline (speedup 1.0000x reference)
"""Trainium2 Bass kernel for nn_DecoderPreLN2 (4-layer cross-attention decoder).

Sharding: data-parallel over batch N=8 across 8 NeuronCores (1 element/core).
Per-core dataflow is fully "transposed" (D-major): activations live as x^T
[D=1024 partitions(8 tiles), T free] so every matmul keeps weights stationary
and no on-device transposes are needed.

v2 design (software-pipelined, bf16):
  - All matmul operands bf16 (psum fp32). ~50x rel-err margin vs the gate.
  - Layer pipelining: K/V projections of layer i+1 are emitted as filler
    chunks INSIDE layer i's attention loop, so the in-order PE queue never
    stalls on the EXP(softmax)-bound attention dependency chain. kt/vaug
    are double-buffered (ping-pong) across layers to permit this.
  - Q projection runs on the RAW (un-normalized) x; LayerNorm is applied
    post-hoc as a per-token affine correction:
      q = rstd*(wq2^T(x+bvp)) - mu*rstd*colsum(wq2) + bq2
    This removes the LN-stats -> smalls -> xn serial chain from the PE
    critical path at each layer boundary.
  - LN stats: one 2-col stationary [ones | 2*bvp] matmul group gives
    sums+cross in a single pass; m2 from squared tiles (8+8 MMs total).
  - Scores computed transposed [k, q] in head pairs at PE row groups
    0-63/64-127 (concurrent via tile_position auto-derive); softmax
    denominator via a ones-column appended to V; mask enters as the exp
    bias; no max subtraction (logits are O(1) by construction).
  - V-bias bv folded into the next layer's LN stats / q-correction
    (softmax weights sum to 1); last layer adds bv explicitly.
"""

import sys

sys.path.insert(0, "/opt/trn_rl_repo")

from collections import deque

import ml_dtypes
import numpy as np

import concourse.bass as bass
import concourse.tile as tile
from concourse import bacc, mybir
from concourse.bass import ts
from concourse.bass_utils import run_bass_kernel_spmd

L, D, H, HD = 4, 1024, 16, 64
TQ, TK, NB = 512, 1024, 8
DT = D // 128  # 8 d-tiles
KT = TK // 128  # 8 k-token tiles
EPS = 1e-5

F32 = mybir.dt.float32
BF16 = mybir.dt.bfloat16
I32 = mybir.dt.int32
AF = mybir.ActivationFunctionType
OP = mybir.AluOpType

_PROGRAM = None


def build_program():
    global _PROGRAM
    if _PROGRAM is not None:
        return _PROGRAM

    nc = bacc.Bacc(
        "TRN2", target_bir_lowering=False, debug=False,
        dynamic_dma_scratch_size=2048,
    )

    xt0 = nc.dram_tensor("xt0", [D, TQ], BF16, kind="ExternalInput").ap()
    enct = nc.dram_tensor("enct", [D, TK], BF16, kind="ExternalInput").ap()
    maskd = nc.dram_tensor("maskd", [128, KT], F32, kind="ExternalInput").ap()
    wqd = nc.dram_tensor("wqd", [L, D, D], BF16, kind="ExternalInput").ap()
    wkd = nc.dram_tensor("wkd", [L, D, D], BF16, kind="ExternalInput").ap()
    wvd = nc.dram_tensor("wvd", [L, D, D], BF16, kind="ExternalInput").ap()
    bq2d = nc.dram_tensor("bq2d", [128, L, DT], F32, kind="ExternalInput").ap()
    bkd = nc.dram_tensor("bkd", [128, L, DT], F32, kind="ExternalInput").ap()
    bvfd = nc.dram_tensor("bvfd", [128, L, DT], F32, kind="ExternalInput").ap()
    wqsd = nc.dram_tensor("wqsd", [128, L, DT], F32, kind="ExternalInput").ap()
    cbvd = nc.dram_tensor("cbvd", [128, L, DT], F32, kind="ExternalInput").ap()
    lncd = nc.dram_tensor("lncd", [1, L, 2], F32, kind="ExternalInput").ap()
    # stats stationary: col 0 = ones (sums), col 32 = 2*bvp (cross term),
    # col 64 = ones again (m2, col-tiled concurrent with the sums matmul).
    # Output rows land at psum partitions {0, 32, 64}, all legal DVE base
    # partitions (partition 1 is not).
    st2d = nc.dram_tensor(
        "st2d", [128, L, DT, 65], BF16, kind="ExternalInput"
    ).ap()
    outd = nc.dram_tensor("outd", [D, TQ], F32, kind="ExternalOutput").ap()

    with tile.TileContext(nc) as tc:
        with (
            tc.tile_pool(name="persist", bufs=1) as persist,
            tc.tile_pool(name="xp", bufs=2) as xp,
            tc.tile_pool(name="wp", bufs=20) as wp,
            tc.tile_pool(name="sqp", bufs=4) as sqp,
            tc.tile_pool(name="ptp", bufs=9) as ptp,
            tc.tile_pool(name="smalls", bufs=2) as smalls,
            tc.tile_pool(name="recipp", bufs=2) as recipp,
            tc.tile_pool(name="bcp", bufs=2) as bcp,
            tc.tile_pool(name="qcp", bufs=2) as qcp,
            tc.tile_pool(name="proj_ps", bufs=2, space="PSUM") as proj_ps,
            tc.tile_pool(name="sc_ps", bufs=2, space="PSUM") as sc_ps,
            tc.tile_pool(name="av_ps", bufs=2, space="PSUM") as av_ps,
        ):
            # ---- persistent tiles ----
            def load_w_half(wd, i, half):
                """Load 8 [128, 512] half-tiles (din-tile k, dout half)."""
                tiles = []
                for k in range(DT):
                    w = wp.tile([128, 512], BF16, tag="w")
                    nc.sync.dma_start(
                        out=w[:], in_=wd[i, ts(k, 128), ts(half, 512)]
                    )
                    tiles.append(w)
                return tiles

            # very first DMAs issued: the weights the first matmul needs
            wk0_pre = load_w_half(wkd, 0, 0)

            enc_r = enct.rearrange("(j p) t -> p j t", p=128)
            enc_sb = persist.tile([128, DT, TK], BF16, tag="enc")
            # fine-grained pieces so the first K-proj matmul only waits on
            # one 128KB transfer, not the whole 2MB
            for k in range(DT):
                for c in range(2):
                    nc.sync.dma_start(
                        out=enc_sb[:, k, ts(c, 512)],
                        in_=enc_r[:, k, ts(c, 512)],
                    )
            kt_sb = [
                persist.tile([128, DT, TK], BF16, tag=f"kt{j}",
                             name=f"kt{j}")
                for j in range(2)
            ]
            vaug_sb = [
                persist.tile([128, KT, H, HD + 1], BF16, tag=f"vaug{j}",
                             name=f"vaug{j}")
                for j in range(2)
            ]
            qt_sb = persist.tile([128, DT, TQ], BF16, tag="qt")
            bk_al = persist.tile([128, L, DT], F32, tag="bk")
            nc.sync.dma_start(out=bk_al[:], in_=bkd[:])
            mask_sb = persist.tile([128, KT], F32, tag="mask")
            bq_al = persist.tile([128, L, DT], F32, tag="bq")
            bvf_al = persist.tile([128, L, DT], F32, tag="bvf")
            wqs_al = persist.tile([128, L, DT], F32, tag="wqs")
            cbv_al = persist.tile([128, L, DT], F32, tag="cbv")
            lnc_al = persist.tile([1, L, 2], F32, tag="lnc")
            st2_al = persist.tile([128, L, DT, 65], BF16, tag="st2")
            onesf = persist.tile([128, H], BF16, tag="onesf")
            rstd_bc = persist.tile([128, TQ], F32, tag="rstd_bc")
            murstd_bc = persist.tile([128, TQ], F32, tag="murstd_bc")
            xout = persist.tile([128, DT, TQ], F32, tag="xout")
            x_cur = xp.tile([128, DT, TQ], BF16, tag="x")

            def load_consts_and_x0():
                """Issued after the first K-proj weight DMAs so the PE can
                start ~10us earlier; nothing here is needed until the V
                copies / layer-0 stats."""
                nc.sync.dma_start(out=mask_sb[:], in_=maskd[:])
                nc.sync.dma_start(out=bq_al[:], in_=bq2d[:])
                nc.sync.dma_start(out=bvf_al[:], in_=bvfd[:])
                nc.sync.dma_start(out=wqs_al[:], in_=wqsd[:])
                nc.sync.dma_start(out=cbv_al[:], in_=cbvd[:])
                nc.sync.dma_start(out=lnc_al[:], in_=lncd[:])
                nc.sync.dma_start(out=st2_al[:], in_=st2d[:])
                # ones column of v_aug: written once, never overwritten
                nc.vector.memset(onesf[:], 1.0)
                for j in range(2):
                    for m in range(KT):
                        nc.vector.tensor_copy(
                            vaug_sb[j][:, m, :, HD : HD + 1], onesf[:]
                        )
                # layer-0 x
                nc.sync.dma_start(
                    out=x_cur[:], in_=xt0.rearrange("(j p) t -> p j t", p=128)
                )

            def kv_gen(i):
                """Yield after each K/V projection psum group of layer i.
                Writes the (i%2) kt/vaug ping-pong buffers."""
                kt_dst = kt_sb[i % 2]
                vg = vaug_sb[i % 2]
                for half in range(2):
                    if i == 0 and half == 0:
                        wk_t = wk0_pre
                    else:
                        wk_t = load_w_half(wkd, i, half)
                    for nl in range(4):
                        n = half * 4 + nl
                        for c in range(2):
                            ps = proj_ps.tile([128, 512], F32, tag="proj")
                            for k in range(DT):
                                nc.tensor.matmul(
                                    ps[:], wk_t[k][:, ts(nl, 128)],
                                    enc_sb[:, k, ts(c, 512)],
                                    start=(k == 0), stop=(k == DT - 1),
                                )
                            # alternate drain engine so neither queue's
                            # backlog holds the psum slot (PE stalls ~650ns
                            # per late drain otherwise)
                            if c == 0:
                                nc.vector.tensor_scalar_add(
                                    kt_dst[:, n, ts(c, 512)], ps[:],
                                    bk_al[:, i, n : n + 1],
                                )
                            else:
                                nc.scalar.activation(
                                    kt_dst[:, n, ts(c, 512)], ps[:],
                                    AF.Identity,
                                    bias=bk_al[:, i, n : n + 1], scale=1.0,
                                )
                            yield
                for half in range(2):
                    wv_t = load_w_half(wvd, i, half)
                    for m in range(KT):
                        ps = proj_ps.tile([128, 512], F32, tag="proj")
                        for k in range(DT):
                            nc.tensor.matmul(
                                ps[:], enc_sb[:, k, ts(m, 128)], wv_t[k][:],
                                start=(k == 0), stop=(k == DT - 1),
                            )
                        # alternate ACT/DVE (Copy is resident in every ACT
                        # table set — no swap)
                        if m % 2 == 0:
                            nc.scalar.copy(
                                vg[:, m, ts(half, 8), 0:HD],
                                ps[:].rearrange("p (h e) -> p h e", h=8),
                            )
                        else:
                            nc.vector.tensor_copy(
                                vg[:, m, ts(half, 8), 0:HD],
                                ps[:].rearrange("p (h e) -> p h e", h=8),
                            )
                        yield

            fillers = deque()  # entries: [generator, remaining_groups]

            def fill(n=1, keep=0):
                """Pull up to n filler chunks, but leave `keep` groups
                pending (reserved for the next layer boundary, where the
                serial LN-smalls chain would otherwise starve the PE)."""
                done = 0
                while done < n and fillers:
                    if keep and sum(e[1] for e in fillers) <= keep:
                        return
                    e = fillers[0]
                    try:
                        next(e[0])
                        e[1] -= 1
                        done += 1
                    except StopIteration:
                        fillers.popleft()

            # prologue: K(0) all 16 groups first (their weight DMAs go out
            # ahead of the const/x0 loads), then consts, then V(0) half 0
            fillers.append([kv_gen(0), 32])
            fill(16)
            load_consts_and_x0()
            fill(8)

            for i in range(L):
                if i + 1 < L:
                    fillers.append([kv_gen(i + 1), 32])
                kt_cur = kt_sb[i % 2]
                vg = vaug_sb[i % 2]
                last = i == L - 1

                # ---- LN statistics (PE reduction over partitions) ----
                # sums/cross (stationary cols 0/32) and m2 (col 64) are
                # col-tiled into the same psum bank and run concurrently.
                sums_ps = proj_ps.tile([65, TQ], F32, tag="proj")
                for k in range(DT):
                    sq = sqp.tile([128, TQ], BF16, tag="sq")
                    # split the square chain across ACT/DVE so the m2
                    # matmuls aren't paced by one engine's serial queue
                    if k % 2 == 0:
                        nc.scalar.square(sq[:], x_cur[:, k, :])
                    else:
                        nc.vector.tensor_mul(
                            sq[:], x_cur[:, k, :], x_cur[:, k, :]
                        )
                    nc.tensor.matmul(
                        sums_ps[0:33, :], st2_al[:, i, k, 0:33],
                        x_cur[:, k, :],
                        start=(k == 0), stop=(k == DT - 1),
                        skip_group_check=True,
                    )
                    nc.tensor.matmul(
                        sums_ps[64:65, :], st2_al[:, i, k, 64:65], sq[:],
                        start=(k == 0), stop=(k == DT - 1),
                        skip_group_check=True,
                    )

                # reserved filler groups cover the PE while the serial
                # LN-smalls chain runs on DVE
                fill(6)

                # ---- LN smalls ----
                mu = smalls.tile([1, TQ], F32, tag="mu")
                nc.vector.tensor_scalar(
                    mu[:], sums_ps[0:1, :], lnc_al[0:1, i, 0:1], 1.0 / D,
                    op0=OP.add, op1=OP.mult,
                )
                cross = smalls.tile([1, TQ], F32, tag="cross")
                nc.vector.tensor_copy(cross[:], sums_ps[32:33, :])
                veps = smalls.tile([1, TQ], F32, tag="veps")
                nc.vector.tensor_scalar(
                    veps[:], sums_ps[64:65, :], lnc_al[0:1, i, 1:2], 1.0 / D,
                    op0=OP.add, op1=OP.mult,
                )
                nc.vector.scalar_tensor_tensor(
                    veps[:], cross[:], 1.0 / D, veps[:],
                    op0=OP.mult, op1=OP.add,
                )
                musq = smalls.tile([1, TQ], F32, tag="musq")
                nc.vector.tensor_mul(musq[:], mu[:], mu[:])
                nc.vector.scalar_tensor_tensor(
                    veps[:], musq[:], -1.0, veps[:], op0=OP.mult, op1=OP.add
                )
                nc.vector.tensor_scalar_add(veps[:], veps[:], EPS)
                # rstd via Quake rsqrt seed + 2 Newton steps, all on DVE
                # (ACT Sqrt would force 2 table-set swaps per layer, since
                # sqrt and exp never share an ACT table set)
                t1 = smalls.tile([1, TQ], F32, tag="t1")
                iv = smalls.tile([1, TQ], I32, tag="iv")
                nc.vector.tensor_scalar(
                    iv[:], veps[:].bitcast(I32), 1, None,
                    op0=OP.logical_shift_right,
                )
                nc.vector.tensor_scalar(
                    t1[:].bitcast(I32), iv[:], -1, 0x5F3759DF,
                    op0=OP.mult, op1=OP.add,
                )
                tmp = smalls.tile([1, TQ], F32, tag="tmp")
                for _ in range(2):
                    nc.vector.tensor_mul(tmp[:], t1[:], t1[:])
                    nc.vector.scalar_tensor_tensor(
                        tmp[:], tmp[:], -0.5, veps[:], op0=OP.mult,
                        op1=OP.mult,
                    )
                    nc.vector.tensor_scalar_add(tmp[:], tmp[:], 1.5)
                    nc.vector.tensor_mul(t1[:], t1[:], tmp[:])  # t1 = rstd
                nc.vector.scalar_tensor_tensor(
                    mu[:], mu[:], -1.0, t1[:], op0=OP.mult, op1=OP.mult
                )  # mu = -mu*rstd
                nc.gpsimd.partition_broadcast(rstd_bc[:], t1[:])
                nc.gpsimd.partition_broadcast(murstd_bc[:], mu[:])

                # ---- Q projection on RAW x + per-token LN correction ----
                # emitted per d-tile inside the attention loop, one head
                # pair ahead of the scores that consume it
                wq_t = {}

                def q_proj(n):
                    half, nl = divmod(n, 4)
                    if nl == 0:
                        wq_t[half] = load_w_half(wqd, i, half)
                    ps = proj_ps.tile([128, 512], F32, tag="proj")
                    for k in range(DT):
                        nc.tensor.matmul(
                            ps[:], wq_t[half][k][:, ts(nl, 128)],
                            x_cur[:, k, :],
                            start=(k == 0), stop=(k == DT - 1),
                        )
                    # tcorr = murstd*wqsum + rstd*cbv + bq2
                    tcorr = qcp.tile([128, TQ], F32, tag="tcorr")
                    nc.vector.tensor_scalar(
                        tcorr[:], rstd_bc[:], cbv_al[:, i, n : n + 1],
                        bq_al[:, i, n : n + 1], op0=OP.mult, op1=OP.add,
                    )
                    nc.vector.scalar_tensor_tensor(
                        tcorr[:], murstd_bc[:], wqs_al[:, i, n : n + 1],
                        tcorr[:], op0=OP.mult, op1=OP.add,
                    )
                    nc.vector.tensor_mul(qt_sb[:, n, :], ps[:], rstd_bc[:])
                    nc.vector.tensor_tensor(
                        qt_sb[:, n, :], qt_sb[:, n, :], tcorr[:], op=OP.add
                    )

                q_proj(0)

                # ---- attention, by head pair (2hp, 2hp+1) ----
                # two heads at PE row groups 0-63 / 64-127 run concurrently
                if last:
                    xnext = xout
                else:
                    xnext = xp.tile([128, DT, TQ], BF16, tag="x")
                for hp in range(DT):
                    hA, hB = 2 * hp, 2 * hp + 1
                    pts = []
                    for kt in range(KT):
                        sc = sc_ps.tile([128, 2, TQ], F32, tag="sc")
                        nc.tensor.matmul(
                            sc[:, 0, :],
                            kt_cur[0:64, hp, ts(kt, 128)],
                            qt_sb[0:64, hp, :],
                            start=True, stop=True,
                        )
                        nc.tensor.matmul(
                            sc[:, 1, :],
                            kt_cur[64:128, hp, ts(kt, 128)],
                            qt_sb[64:128, hp, :],
                            start=True, stop=True,
                        )
                        pt = ptp.tile([128, 2, TQ], BF16, tag="pt")
                        nc.scalar.activation(
                            pt[:], sc[:], AF.Exp,
                            bias=mask_sb[:, kt : kt + 1], scale=1.0,
                        )
                        pts.append(pt)
                        if kt in (1, 3, 5):
                            fill(1, keep=6)
                    if hp + 1 < DT:
                        q_proj(hp + 1)
                    fill(1, keep=6)
                    avA = av_ps.tile([HD + 1, TQ], F32, tag="av")
                    avB = av_ps.tile([HD + 1, TQ], F32, tag="av")
                    for kt in range(KT):
                        nc.tensor.matmul(
                            avA[:], vg[:, kt, hA, :], pts[kt][:, 0, :],
                            start=(kt == 0), stop=(kt == KT - 1),
                        )
                        nc.tensor.matmul(
                            avB[:], vg[:, kt, hB, :], pts[kt][:, 1, :],
                            start=(kt == 0), stop=(kt == KT - 1),
                        )
                        if kt == 3:
                            fill(1, keep=6)
                    fill(1, keep=6)
                    for av, o, h in ((avA, 0, hA), (avB, 64, hB)):
                        # realign denom row to base partition 0:
                        # reciprocal_approx_fast breaks on bp!=0 inputs
                        recip = recipp.tile([1, TQ], F32, tag="recip")
                        nc.vector.tensor_copy(recip[:], av[HD : HD + 1, :])
                        nc.vector.reciprocal_approx_fast(recip[:], recip[:])
                        bc = bcp.tile([64, TQ], F32, tag="bc")
                        nc.gpsimd.partition_broadcast(bc[:], recip[:])
                        nc.vector.tensor_tensor(
                            xnext[o : o + 64, hp, :], av[0:HD, :], bc[:],
                            op=OP.mult,
                        )
                        if last:
                            nc.vector.tensor_scalar_add(
                                xnext[o : o + 64, hp, :],
                                xnext[o : o + 64, hp, :],
                                bvf_al[o : o + 64, i, hp : hp + 1],
                            )
                            # stream each finished half-column out early
                            nc.sync.dma_start(
                                out=outd.rearrange(
                                    "(j p) t -> p j t", p=128
                                )[o : o + 64, hp, :],
                                in_=xout[o : o + 64, hp, :],
                            )
                x_cur = xnext

    nc.compile()
    _PROGRAM = nc
    return nc


def _to_bf16(a):
    return np.asarray(a, dtype=np.float32).astype(ml_dtypes.bfloat16)


def _stage_inputs(input_ids, encoder_state, cross_attn_mask, emb,
                  ln_g, ln_b, wq, bq, wk, bk, wv, bv):
    input_ids = np.asarray(input_ids)
    emb = np.asarray(emb, dtype=np.float32)
    encoder_state = np.asarray(encoder_state, dtype=np.float32)
    cross_attn_mask = np.asarray(cross_attn_mask, dtype=np.float32)
    ln_g = np.asarray(ln_g, dtype=np.float32)
    ln_b = np.asarray(ln_b, dtype=np.float32)
    wq = np.asarray(wq, dtype=np.float32)
    bq = np.asarray(bq, dtype=np.float32)
    wk = np.asarray(wk, dtype=np.float32)
    bk = np.asarray(bk, dtype=np.float32)
    wv = np.asarray(wv, dtype=np.float32)
    bv = np.asarray(bv, dtype=np.float32)

    scale = 1.0 / np.sqrt(HD)
    # fold LN affine + scores scale into wq/bq
    wq2 = ln_g[:, :, None] * wq * scale  # [L, D, D]
    bq2 = (np.einsum("ld,lde->le", ln_b, wq) + bq) * scale  # [L, D]

    bv_prev = np.concatenate([np.zeros((1, D), np.float32), bv[:-1]], axis=0)
    wqsum = wq2.sum(axis=1)  # [L, D] (over d_in)
    cbv = np.einsum("ld,lde->le", bv_prev, wq2)  # [L, D]
    lnc = np.stack(
        [bv_prev.sum(axis=1), (bv_prev * bv_prev).sum(axis=1)], axis=1
    ).astype(np.float32)[None, :, :]  # [1, L, 2]

    def pcol(a):  # [L, D] -> [128, L, DT] with a[l, j*128+p] at [p, l, j]
        return np.ascontiguousarray(a.reshape(L, DT, 128).transpose(2, 0, 1))

    st2 = np.zeros((128, L, DT, 65), dtype=ml_dtypes.bfloat16)
    st2[..., 0] = np.float32(1.0)
    st2[..., 32] = pcol(2.0 * bv_prev).astype(ml_dtypes.bfloat16)
    st2[..., 64] = np.float32(1.0)

    shared = {
        "wqd": _to_bf16(wq2),
        "wkd": _to_bf16(wk),
        "wvd": _to_bf16(wv),
        "bq2d": pcol(bq2),
        "bkd": pcol(bk),
        "bvfd": pcol(bv),
        "wqsd": pcol(wqsum),
        "cbvd": pcol(cbv),
        "lncd": np.ascontiguousarray(lnc),
        "st2d": st2,
    }

    x0 = emb[input_ids]  # [NB, TQ, D]
    in_maps = []
    for n in range(NB):
        m = dict(shared)
        m["xt0"] = _to_bf16(x0[n].T)
        m["enct"] = _to_bf16(encoder_state[n].T)
        m["maskd"] = np.ascontiguousarray(
            cross_attn_mask[n, 0, 0].reshape(KT, 128).T
        )
        in_maps.append(m)
    return in_maps


def kernel(**inputs) -> np.ndarray:
    nc = build_program()
    in_maps = _stage_inputs(**inputs)
    res = run_bass_kernel_spmd(nc, in_maps, list(range(NB)))
    out = np.stack([np.asarray(res.results[n]["outd"]).T for n in range(NB)])
    return np.ascontiguousarray(out, dtype=np.float32)


if __name__ == "__main__":
    build_program()
    print("program built ok")
